# revision 40
# baseline (speedup 1.0000x reference)
"""Chamfer distance kernel for Trainium2 (Bass/Tile), SPMD over 8 NeuronCores.

Math (per batch b):
  dist[v,l] = ||x_v||^2 - 2 x_v.y_l + ||y_l||^2,  x=[1024,512], y=[512,512]
  out[b] = mean_v min_l dist + mean_l min_v dist

Strategy v11 (transposed layout + exp/LSE D1 + exact-min D2):
  - Data-parallel over batch: 64 batches -> 8 cores x 8 batches.
  - LAYOUT: l (lang, 512) on PSUM partitions, v (video) on the free dim.
    Per batch: 4 l-chunks x 2 v-halves of [128, 512] = dist^T tiles.
  - PE per (c,h) tile: 2 fp8 DoubleRow matmuls (K=512) + one K=3 aug
    matmul whose rows carry a_v = ||x_q||^2 as fp8 (hi/64, mid, lo
    residuals; err ~ +-0.13); the 4 augs of a c-pair are row-packed 4x
    via tile_position (rows 0/32/64/96 run concurrently). pm = q + a_v.
  - ACT per c-chunk (fused [128, 2, 512]): w = exp(-beta*pm +
    beta*(SHIFT - b_l)) = exp(beta*(SHIFT - dist)) in bf16; b_l rides
    the per-partition bias EXACTLY (f32).
  - DVE per c-chunk: tensor_tensor(max) then tensor_reduce(max) ->
    exact min_v dist per l (D2).
  - PE: 8 ones-vector matmuls, col-packed 4x via tile_position, all
    into ONE PSUM bank (c and c+2 accumulate into the same row) ->
    ssum = sum_l w -> soft-min over l per v (D1, LSE).  Emitted one
    batch late so the PE never waits on ACT.
  - Warmup matmuls on memset scratch un-throttle the PE (HAM) during
    the input-DMA lead-in; aux DMAs ride the idle GpSimd SWDGE queue;
    outputs for batches 0..5 stream out during batch 7.
  - Host: d1_v = SHIFT - ln(sum rows ssum)/beta (softmin), d2_l =
    SHIFT - ln(maxw)/beta (exact); out = mean(d1) + mean(d2), in f64.
"""

import numpy as np

N_CORES = 8
B = 8          # batches per core
D = 512        # feature dim
NV = 1024      # video clips
NL = 512       # language tokens
P = 128        # partitions
KC = D // P    # contraction chunks = 4
CL = NL // P   # l chunks = 4
H = NV // 512  # v halves = 2

BETA = 0.25    # LSE sharpness for the D1 softmin
SHIFT = 900.0  # exp arg = beta*(SHIFT - dist); max arg ~ 52 << fp32's 88

N_WARM = 9     # PE warmup matmuls (HAM un-throttle during DMA lead-in)

_CACHE = {}


def _build_bass():
    import concourse.bass as bass
    import concourse.mybir as mybir
    import concourse.tile as tile
    from concourse import bacc

    f32 = mybir.dt.float32
    bf16 = mybir.dt.bfloat16
    f8 = mybir.dt.float8e4
    ALU = mybir.AluOpType
    AFT = mybir.ActivationFunctionType
    DR = mybir.MatmulPerfMode.DoubleRow

    nc = bacc.Bacc(None)
    xs_h = nc.declare_dram_parameter("xs", [B, P, KC, NV], f8, isOutput=False)
    ys_h = nc.declare_dram_parameter("ys", [B, P, KC, NL], f8, isOutput=False)
    am_h = nc.declare_dram_parameter("am3", [B, 3, NV + P], f8, isOutput=False)
    bt_h = nc.declare_dram_parameter("bt", [P, B, CL], f32, isOutput=False)
    on_h = nc.declare_dram_parameter("ones", [P, 1], bf16, isOutput=False)
    ss_h = nc.declare_dram_parameter("ssum", [4, B, 512], f32, isOutput=True)
    d2_h = nc.declare_dram_parameter("d2c", [P, B, CL], f32, isOutput=True)

    def emit_ones(ps, on_t, prev):
        """D1 ones-matmuls (partition sums of w) + PSUM->SBUF copy.

        8 jobs (c,h) share ONE PSUM bank: col slot 32*(2*(c%2)+h); c and
        c+2 accumulate into the same row (host sums rows anyway).  Two
        4-way col-packed concurrent groups.
        """
        ws, b, cs, bi = prev
        ss_t = ps.tile([P, 512], f32, tag="ss", bufs=2)
        for grp in range(2):
            for cm in range(2):
                for h in range(H):
                    c = 2 * grp + cm
                    slot = 32 * (2 * cm + h)
                    nc.tensor.matmul(
                        out=ss_t[slot : slot + 1, :],
                        lhsT=on_t,
                        rhs=ws[c][:, h, :],
                        start=(grp == 0),
                        stop=(grp == 1),
                        tile_position=(0, slot),
                    )
        nc.vector.tensor_copy(out=cs[:, bi], in_=ss_t[0:97])

    with tile.TileContext(nc) as tc:
        with (
            tc.tile_pool(name="cst", bufs=1) as cst,
            tc.tile_pool(name="io", bufs=2) as io,
            tc.tile_pool(name="wp", bufs=9) as wp,
            tc.tile_pool(name="out", bufs=1) as op_,
            tc.tile_pool(name="ps", bufs=1, space="PSUM") as ps,
        ):
            # PE warmup: memset scratch, then just enough matmuls to
            # un-throttle HAM while the first input DMAs are in flight.
            wst = cst.tile([P, 1], bf16, tag="wst")
            wsc = cst.tile([P, 512], bf16, tag="wsc")
            nc.vector.memset(wst, 1.0)
            nc.vector.memset(wsc, 1.0)
            warm = ps.tile([P, H, 512], f32, tag="pm", bufs=3)
            for i in range(N_WARM):
                nc.tensor.matmul(
                    out=warm[0:1, 0, :], lhsT=wst, rhs=wsc, start=True, stop=True
                )

            # Batch-0 inputs first, split across BOTH HWDGE queues (sync +
            # scalar) so the first matmul's operands land ASAP.  The aug
            # tensor carries both the a-rows (cols 0:NV) and the constant
            # stationary rows (cols NV:NV+P), one DMA per row-group.
            xs0 = io.tile([P, KC, NV], f8, tag="xs")
            ys0 = io.tile([P, KC, NL], f8, tag="ys")
            am0 = io.tile([P, NV + P], f8, tag="am")
            nc.scalar.dma_start(out=xs0[:, :2], in_=xs_h[0, :, :2])
            nc.sync.dma_start(out=ys0, in_=ys_h[0])
            nc.sync.dma_start(out=xs0[:, 2:], in_=xs_h[0, :, 2:])

            on_t = cst.tile([P, 1], bf16, tag="on")     # ones column
            bt_t = cst.tile([P, B, CL], f32, tag="bt")  # ACT bias beta*(SHIFT-b_l)
            for g in range(4):
                nc.gpsimd.dma_start(out=am0[32 * g : 32 * g + 3], in_=am_h[0])
            nc.scalar.dma_start(out=bt_t, in_=bt_h[:])
            nc.gpsimd.dma_start(out=on_t, in_=on_h[:])

            # Whole-kernel accumulators, split so batches 0..6 can stream
            # out (clean deps) while batch 7 still computes.
            d2a = op_.tile([P, B - 1, CL], f32, tag="d2a")
            d2b = op_.tile([P, 1, CL], f32, tag="d2b")
            csa = op_.tile([97, B - 1, 512], f32, tag="csa")
            csb = op_.tile([97, 1, 512], f32, tag="csb")

            prev = None  # deferred ones-matmul work for the previous batch
            for b in range(B):
                if b == 0:
                    xs_t, ys_t, am_t = xs0, ys0, am0
                else:
                    xs_t = io.tile([P, KC, NV], f8, tag="xs")
                    ys_t = io.tile([P, KC, NL], f8, tag="ys")
                    am_t = io.tile([P, NV + P], f8, tag="am")
                    nc.sync.dma_start(out=ys_t, in_=ys_h[b])
                    nc.sync.dma_start(out=xs_t, in_=xs_h[b])
                    for g in range(4):
                        nc.gpsimd.dma_start(
                            out=am_t[32 * g : 32 * g + 3], in_=am_h[b]
                        )
                d2t, d2i = (d2a, b) if b < B - 1 else (d2b, 0)
                cst_, csi = (csa, b) if b < B - 1 else (csb, 0)
                ws = []
                for cp in range(2):  # c-pairs
                    pms = []
                    for dc in range(2):
                        pm2 = ps.tile([P, H, 512], f32, tag="pm", bufs=3)
                        pms.append(pm2)
                        c = 2 * cp + dc
                        for kk in range(2):
                            for h in range(H):
                                nc.tensor.matmul(
                                    out=pm2[:, h, :],
                                    lhsT=ys_t[:, 2 * kk : 2 * kk + 2, c * P : (c + 1) * P],
                                    rhs=xs_t[:, 2 * kk : 2 * kk + 2, h * 512 : (h + 1) * 512],
                                    start=(kk == 0),
                                    stop=False,
                                    perf_mode=DR,
                                )
                    # Row-packed (4x) aug matmuls add a_v via 3 fp8 rows.
                    for g in range(4):
                        dc, h = divmod(g, 2)
                        nc.tensor.matmul(
                            out=pms[dc][:, h, :],
                            lhsT=am_t[32 * g : 32 * g + 3, NV : NV + P],
                            rhs=am_t[32 * g : 32 * g + 3, h * 512 : (h + 1) * 512],
                            start=False,
                            stop=True,
                            tile_position=(32 * g, 0),
                        )
                    for dc in range(2):
                        c = 2 * cp + dc
                        # w = exp(beta*(SHIFT - dist)), fused over both halves.
                        w2 = wp.tile([P, H, 512], bf16, tag="w")
                        ws.append(w2)
                        nc.scalar.activation(
                            out=w2,
                            in_=pms[dc],
                            func=AFT.Exp,
                            bias=bt_t[:, b, c : c + 1],
                            scale=-BETA,
                        )
                        # D2 exact: d2c = max over (h, v) of w per l.
                        wm = wp.tile([P, 512], bf16, tag="wm", bufs=3)
                        nc.vector.tensor_tensor(
                            out=wm, in0=w2[:, 0, :], in1=w2[:, 1, :], op=ALU.max
                        )
                        nc.vector.tensor_reduce(
                            out=d2t[:, d2i, c : c + 1],
                            in_=wm,
                            axis=mybir.AxisListType.X,
                            op=ALU.max,
                        )
                    # Deferred D1 ones-matmuls for the previous batch.
                    if cp == 1 and prev is not None:
                        emit_ones(ps, on_t, prev)
                        # Stream batches 0..6 out on GpSimd while batch 7
                        # finishes; both accumulators are now complete.
                        if b == B - 1:
                            nc.gpsimd.dma_start(out=d2_h[:, 0 : B - 1], in_=d2a)
                            for g in range(4):
                                nc.gpsimd.dma_start(
                                    out=ss_h[g : g + 1, 0 : B - 1],
                                    in_=csa[32 * g : 32 * g + 1],
                                )
                prev = (ws, b, cst_, csi)
            nc.sync.dma_start(out=d2_h[:, B - 1 :], in_=d2b)
            emit_ones(ps, on_t, prev)

            # Final slivers fan out across both HWDGE queues.
            nc.sync.dma_start(out=ss_h[0:1, B - 1 :], in_=csb[0:1])
            nc.scalar.dma_start(out=ss_h[1:2, B - 1 :], in_=csb[32:33])
            nc.sync.dma_start(out=ss_h[2:3, B - 1 :], in_=csb[64:65])
            nc.scalar.dma_start(out=ss_h[3:4, B - 1 :], in_=csb[96:97])

    nc.finalize()
    return nc


def _get_bass():
    if "nc" not in _CACHE:
        _CACHE["nc"] = _build_bass()
    return _CACHE["nc"]


def _run(in_maps, trace=False):
    from concourse.bass_utils import run_bass_kernel_spmd

    nc = _get_bass()
    return run_bass_kernel_spmd(nc, in_maps, list(range(N_CORES)), trace=trace)


def make_in_maps(video_feat, lang_feat):
    import ml_dtypes

    f8 = ml_dtypes.float8_e4m3
    bf16 = ml_dtypes.bfloat16
    video = np.asarray(video_feat, dtype=np.float32)
    lang = np.asarray(lang_feat, dtype=np.float32)
    assert video.shape == (N_CORES * B, NV, D), video.shape
    assert lang.shape == (N_CORES * B, NL, D), lang.shape
    NB = N_CORES * B

    xs8 = (-2.0 * video).astype(f8)                      # [64, NV, D]
    ys8 = lang.astype(f8)                                # [64, NL, D]
    xsf = xs8.astype(np.float32)
    ysf = ys8.astype(np.float32)
    a = np.einsum("bvd,bvd->bv", xsf, xsf) / 4.0         # ||x_q||^2  [64, NV]
    bn = np.einsum("bld,bld->bl", ysf, ysf)              # ||y_q||^2  [64, NL]

    # a_v as 3 fp8 aug rows: a ~= 64*hi + mid + lo (err ~ +-0.13).
    a_hi = (a / 64.0).astype(f8)
    r1 = a - 64.0 * a_hi.astype(np.float32)
    a_mid = r1.astype(f8)
    a_lo = (r1 - a_mid.astype(np.float32)).astype(f8)
    am3 = np.zeros((NB, 3, NV + P), f8)                  # a-rows | consts
    am3[:, 0, :NV] = a_hi
    am3[:, 1, :NV] = a_mid
    am3[:, 2, :NV] = a_lo
    am3[:, 0, NV:] = np.float32(64.0)
    am3[:, 1, NV:] = np.float32(1.0)
    am3[:, 2, NV:] = np.float32(1.0)

    # ACT bias: beta*(SHIFT - b_l), laid out [P, B, CL] per core.
    bt = (BETA * (SHIFT - bn)).astype(np.float32)        # [64, NL]
    bt = bt.reshape(NB, CL, P).transpose(2, 0, 1)        # [P, 64, CL]

    ones = np.ones((P, 1), bf16)

    xs_dev = np.ascontiguousarray(
        xs8.reshape(NB, NV, KC, P).transpose(0, 3, 2, 1)
    )  # [64, P, KC, NV]
    ys_dev = np.ascontiguousarray(
        ys8.reshape(NB, NL, KC, P).transpose(0, 3, 2, 1)
    )  # [64, P, KC, NL]

    in_maps = []
    for cidx in range(N_CORES):
        sl = slice(cidx * B, (cidx + 1) * B)
        in_maps.append(
            {
                "xs": xs_dev[sl],
                "ys": ys_dev[sl],
                "am3": np.ascontiguousarray(am3[sl]),
                "bt": np.ascontiguousarray(bt[:, sl]),
                "ones": ones,
            }
        )
    return in_maps


def finish(res):
    """Host finish in f64: d1 soft-min per v from ssum, d2 exact per l.

    ssum rows r = 2*cm + h hold sum over c in {cm, cm+2} of the l-chunk
    partition sums for v-half h; total per v needs rows {h, 2+h}.
    """
    outs = []
    for cidx in range(N_CORES):
        ss = res.results[cidx]["ssum"].astype(np.float64)  # [4, B, 512]
        d2 = res.results[cidx]["d2c"].astype(np.float64)   # [P, B, CL]
        S = np.stack([ss[0] + ss[2], ss[1] + ss[3]], axis=1)  # [B, H, 512]
        d1 = SHIFT - np.log(S) / BETA                      # [B, H, 512]
        d2l = SHIFT - np.log(d2) / BETA                    # [P, B, CL]
        out = d1.mean(axis=(1, 2)) + d2l.mean(axis=(0, 2))
        outs.append(out.astype(np.float32))
    return np.concatenate(outs)


def kernel(video_feat, lang_feat):
    in_maps = make_in_maps(video_feat, lang_feat)
    res = _run(in_maps, trace=False)
    return finish(res).astype(np.float32)


# revision 42
# speedup vs baseline: 1.1032x; 1.1032x over previous
"""Chamfer distance kernel for Trainium2 (Bass/Tile), SPMD over 8 NeuronCores.

Math (per batch b):
  dist[v,l] = ||x_v||^2 - 2 x_v.y_l + ||y_l||^2,  x=[1024,512], y=[512,512]
  out[b] = mean_v min_l dist + mean_l min_v dist

Strategy v11 (transposed layout + exp/LSE D1 + exact-min D2):
  - Data-parallel over batch: 64 batches -> 8 cores x 8 batches.
  - LAYOUT: l (lang, 512) on PSUM partitions, v (video) on the free dim.
    Per batch: 4 l-chunks x 2 v-halves of [128, 512] = dist^T tiles.
  - PE per (c,h) tile: 2 fp8 DoubleRow matmuls (K=512) + one K=3 aug
    matmul whose rows carry a_v = ||x_q||^2 as fp8 (hi/64, mid, lo
    residuals; err ~ +-0.13); the 4 augs of a c-pair are row-packed 4x
    via tile_position (rows 0/32/64/96 run concurrently). pm = q + a_v.
  - ACT per c-chunk (fused [128, 2, 512]): w = exp(-beta*pm +
    beta*(SHIFT - b_l)) = exp(beta*(SHIFT - dist)) in bf16; b_l rides
    the per-partition bias EXACTLY (f32).
  - DVE per c-chunk: tensor_tensor(max) then tensor_reduce(max) ->
    exact min_v dist per l (D2).
  - PE: 8 ones-vector matmuls, col-packed 4x via tile_position, all
    into ONE PSUM bank (c and c+2 accumulate into the same row) ->
    ssum = sum_l w -> soft-min over l per v (D1, LSE).  Emitted one
    batch late so the PE never waits on ACT.
  - Warmup matmuls on memset scratch un-throttle the PE (HAM) during
    the input-DMA lead-in; aux DMAs ride the idle GpSimd SWDGE queue;
    outputs for batches 0..5 stream out during batch 7.
  - Host: d1_v = SHIFT - ln(sum rows ssum)/beta (softmin), d2_l =
    SHIFT - ln(maxw)/beta (exact); out = mean(d1) + mean(d2), in f64.
"""

import numpy as np

N_CORES = 8
B = 8          # batches per core
D = 512        # feature dim
NV = 1024      # video clips
NL = 512       # language tokens
P = 128        # partitions
KC = D // P    # contraction chunks = 4
CL = NL // P   # l chunks = 4
H = NV // 512  # v halves = 2

BETA = 0.25    # LSE sharpness for the D1 softmin
SHIFT = 900.0  # exp arg = beta*(SHIFT - dist); max arg ~ 52 << fp32's 88

N_WARM = 9     # PE warmup matmuls (HAM un-throttle during DMA lead-in)

_CACHE = {}


def _build_bass():
    import concourse.bass as bass
    import concourse.mybir as mybir
    import concourse.tile as tile
    from concourse import bacc

    f32 = mybir.dt.float32
    bf16 = mybir.dt.bfloat16
    f8 = mybir.dt.float8e4
    ALU = mybir.AluOpType
    AFT = mybir.ActivationFunctionType
    DR = mybir.MatmulPerfMode.DoubleRow

    nc = bacc.Bacc(None)
    xs_h = nc.declare_dram_parameter("xs", [B, P, KC, NV], f8, isOutput=False)
    ys_h = nc.declare_dram_parameter("ys", [B, P, KC, NL], f8, isOutput=False)
    am_h = nc.declare_dram_parameter("am3", [B, 3, NV + P], f8, isOutput=False)
    bt_h = nc.declare_dram_parameter("bt", [P, B, CL], f32, isOutput=False)
    on_h = nc.declare_dram_parameter("ones", [P, 1], bf16, isOutput=False)
    ss_h = nc.declare_dram_parameter("ssum", [4, B, 512], f32, isOutput=True)
    d2_h = nc.declare_dram_parameter("d2c", [P, B, CL], f32, isOutput=True)

    def emit_ones(ps, on_t, prev):
        """D1 ones-matmuls (partition sums of w) + PSUM->SBUF copy.

        8 jobs (c,h) share ONE PSUM bank: col slot 32*(2*(c%2)+h); c and
        c+2 accumulate into the same row (host sums rows anyway).  Two
        4-way col-packed concurrent groups.
        """
        ws, b, cs, bi = prev
        ss_t = ps.tile([P, 512], f32, tag="ss", bufs=2)
        for grp in range(2):
            for cm in range(2):
                for h in range(H):
                    c = 2 * grp + cm
                    slot = 32 * (2 * cm + h)
                    nc.tensor.matmul(
                        out=ss_t[slot : slot + 1, :],
                        lhsT=on_t,
                        rhs=ws[c][:, h, :],
                        start=(grp == 0),
                        stop=(grp == 1),
                        tile_position=(0, slot),
                    )
        nc.vector.tensor_copy(out=cs[:, bi], in_=ss_t[0:97])

    with tile.TileContext(nc) as tc:
        with (
            tc.tile_pool(name="cst", bufs=1) as cst,
            tc.tile_pool(name="io", bufs=2) as io,
            tc.tile_pool(name="wp", bufs=9) as wp,
            tc.tile_pool(name="out", bufs=1) as op_,
            tc.tile_pool(name="ps", bufs=1, space="PSUM") as ps,
        ):
            # PE warmup: memset scratch, then just enough matmuls to
            # un-throttle HAM while the first input DMAs are in flight.
            wst = cst.tile([P, 1], bf16, tag="wst")
            wsc = cst.tile([P, 512], bf16, tag="wsc")
            nc.vector.memset(wst, 1.0)
            nc.vector.memset(wsc, 1.0)
            warm = ps.tile([P, H, 512], f32, tag="pm", bufs=3)
            for i in range(N_WARM):
                nc.tensor.matmul(
                    out=warm[0:1, 0, :], lhsT=wst, rhs=wsc, start=True, stop=True
                )

            # Batch-0 inputs first, split across BOTH HWDGE queues (sync +
            # scalar) so the first matmul's operands land ASAP.  The aug
            # tensor carries both the a-rows (cols 0:NV) and the constant
            # stationary rows (cols NV:NV+P), one DMA per row-group.
            xs0 = io.tile([P, KC, NV], f8, tag="xs")
            ys0 = io.tile([P, KC, NL], f8, tag="ys")
            am0 = io.tile([P, NV + P], f8, tag="am")
            nc.scalar.dma_start(out=xs0[:, :2], in_=xs_h[0, :, :2])
            nc.sync.dma_start(out=ys0, in_=ys_h[0])
            nc.sync.dma_start(out=xs0[:, 2:], in_=xs_h[0, :, 2:])

            on_t = cst.tile([P, 1], bf16, tag="on")     # ones column
            bt_t = cst.tile([P, B, CL], f32, tag="bt")  # ACT bias beta*(SHIFT-b_l)
            for g in range(4):
                nc.sync.dma_start(out=am0[32 * g : 32 * g + 3], in_=am_h[0])
            nc.scalar.dma_start(out=bt_t, in_=bt_h[:])
            nc.gpsimd.dma_start(out=on_t, in_=on_h[:])

            # Whole-kernel accumulators, split so batches 0..6 can stream
            # out (clean deps) while batch 7 still computes.
            d2a = op_.tile([P, B - 1, CL], f32, tag="d2a")
            d2b = op_.tile([P, 1, CL], f32, tag="d2b")
            csa = op_.tile([97, B - 1, 512], f32, tag="csa")
            csb = op_.tile([97, 1, 512], f32, tag="csb")

            prev = None  # deferred ones-matmul work for the previous batch
            for b in range(B):
                if b == 0:
                    xs_t, ys_t, am_t = xs0, ys0, am0
                else:
                    xs_t = io.tile([P, KC, NV], f8, tag="xs")
                    ys_t = io.tile([P, KC, NL], f8, tag="ys")
                    am_t = io.tile([P, NV + P], f8, tag="am")
                    nc.sync.dma_start(out=ys_t, in_=ys_h[b])
                    nc.sync.dma_start(out=xs_t, in_=xs_h[b])
                    for g in range(4):
                        nc.gpsimd.dma_start(
                            out=am_t[32 * g : 32 * g + 3], in_=am_h[b]
                        )
                d2t, d2i = (d2a, b) if b < B - 1 else (d2b, 0)
                cst_, csi = (csa, b) if b < B - 1 else (csb, 0)
                ws = []
                for cp in range(2):  # c-pairs
                    pms = []
                    for dc in range(2):
                        pm2 = ps.tile([P, H, 512], f32, tag="pm", bufs=3)
                        pms.append(pm2)
                        c = 2 * cp + dc
                        for kk in range(2):
                            for h in range(H):
                                nc.tensor.matmul(
                                    out=pm2[:, h, :],
                                    lhsT=ys_t[:, 2 * kk : 2 * kk + 2, c * P : (c + 1) * P],
                                    rhs=xs_t[:, 2 * kk : 2 * kk + 2, h * 512 : (h + 1) * 512],
                                    start=(kk == 0),
                                    stop=False,
                                    perf_mode=DR,
                                )
                    # Row-packed (4x) aug matmuls add a_v via 3 fp8 rows.
                    for g in range(4):
                        dc, h = divmod(g, 2)
                        nc.tensor.matmul(
                            out=pms[dc][:, h, :],
                            lhsT=am_t[32 * g : 32 * g + 3, NV : NV + P],
                            rhs=am_t[32 * g : 32 * g + 3, h * 512 : (h + 1) * 512],
                            start=False,
                            stop=True,
                            tile_position=(32 * g, 0),
                        )
                    for dc in range(2):
                        c = 2 * cp + dc
                        # w = exp(beta*(SHIFT - dist)), fused over both halves.
                        w2 = wp.tile([P, H, 512], bf16, tag="w")
                        ws.append(w2)
                        nc.scalar.activation(
                            out=w2,
                            in_=pms[dc],
                            func=AFT.Exp,
                            bias=bt_t[:, b, c : c + 1],
                            scale=-BETA,
                        )
                        # D2 exact: d2c = max over (h, v) of w per l.
                        wm = wp.tile([P, 512], bf16, tag="wm", bufs=3)
                        nc.vector.tensor_tensor(
                            out=wm, in0=w2[:, 0, :], in1=w2[:, 1, :], op=ALU.max
                        )
                        nc.vector.tensor_reduce(
                            out=d2t[:, d2i, c : c + 1],
                            in_=wm,
                            axis=mybir.AxisListType.X,
                            op=ALU.max,
                        )
                    # Deferred D1 ones-matmuls for the previous batch.
                    if cp == 1 and prev is not None:
                        emit_ones(ps, on_t, prev)
                        # Stream batches 0..6 out while batch 7 finishes;
                        # both accumulators are now complete.  Kept off
                        # GpSimd so its slow dge_drain runs early/hidden.
                        if b == B - 1:
                            nc.sync.dma_start(out=d2_h[:, 0 : B - 1], in_=d2a)
                            for g in range(4):
                                q = nc.sync if g % 2 == 0 else nc.scalar
                                q.dma_start(
                                    out=ss_h[g : g + 1, 0 : B - 1],
                                    in_=csa[32 * g : 32 * g + 1],
                                )
                prev = (ws, b, cst_, csi)
            nc.sync.dma_start(out=d2_h[:, B - 1 :], in_=d2b)
            emit_ones(ps, on_t, prev)

            # Final slivers fan out across both HWDGE queues.
            nc.sync.dma_start(out=ss_h[0:1, B - 1 :], in_=csb[0:1])
            nc.scalar.dma_start(out=ss_h[1:2, B - 1 :], in_=csb[32:33])
            nc.sync.dma_start(out=ss_h[2:3, B - 1 :], in_=csb[64:65])
            nc.scalar.dma_start(out=ss_h[3:4, B - 1 :], in_=csb[96:97])

    nc.finalize()
    return nc


def _get_bass():
    if "nc" not in _CACHE:
        _CACHE["nc"] = _build_bass()
    return _CACHE["nc"]


def _run(in_maps, trace=False):
    from concourse.bass_utils import run_bass_kernel_spmd

    nc = _get_bass()
    return run_bass_kernel_spmd(nc, in_maps, list(range(N_CORES)), trace=trace)


def make_in_maps(video_feat, lang_feat):
    import ml_dtypes

    f8 = ml_dtypes.float8_e4m3
    bf16 = ml_dtypes.bfloat16
    video = np.asarray(video_feat, dtype=np.float32)
    lang = np.asarray(lang_feat, dtype=np.float32)
    assert video.shape == (N_CORES * B, NV, D), video.shape
    assert lang.shape == (N_CORES * B, NL, D), lang.shape
    NB = N_CORES * B

    xs8 = (-2.0 * video).astype(f8)                      # [64, NV, D]
    ys8 = lang.astype(f8)                                # [64, NL, D]
    xsf = xs8.astype(np.float32)
    ysf = ys8.astype(np.float32)
    a = np.einsum("bvd,bvd->bv", xsf, xsf) / 4.0         # ||x_q||^2  [64, NV]
    bn = np.einsum("bld,bld->bl", ysf, ysf)              # ||y_q||^2  [64, NL]

    # a_v as 3 fp8 aug rows: a ~= 64*hi + mid + lo (err ~ +-0.13).
    a_hi = (a / 64.0).astype(f8)
    r1 = a - 64.0 * a_hi.astype(np.float32)
    a_mid = r1.astype(f8)
    a_lo = (r1 - a_mid.astype(np.float32)).astype(f8)
    am3 = np.zeros((NB, 3, NV + P), f8)                  # a-rows | consts
    am3[:, 0, :NV] = a_hi
    am3[:, 1, :NV] = a_mid
    am3[:, 2, :NV] = a_lo
    am3[:, 0, NV:] = np.float32(64.0)
    am3[:, 1, NV:] = np.float32(1.0)
    am3[:, 2, NV:] = np.float32(1.0)

    # ACT bias: beta*(SHIFT - b_l), laid out [P, B, CL] per core.
    bt = (BETA * (SHIFT - bn)).astype(np.float32)        # [64, NL]
    bt = bt.reshape(NB, CL, P).transpose(2, 0, 1)        # [P, 64, CL]

    ones = np.ones((P, 1), bf16)

    xs_dev = np.ascontiguousarray(
        xs8.reshape(NB, NV, KC, P).transpose(0, 3, 2, 1)
    )  # [64, P, KC, NV]
    ys_dev = np.ascontiguousarray(
        ys8.reshape(NB, NL, KC, P).transpose(0, 3, 2, 1)
    )  # [64, P, KC, NL]

    in_maps = []
    for cidx in range(N_CORES):
        sl = slice(cidx * B, (cidx + 1) * B)
        in_maps.append(
            {
                "xs": xs_dev[sl],
                "ys": ys_dev[sl],
                "am3": np.ascontiguousarray(am3[sl]),
                "bt": np.ascontiguousarray(bt[:, sl]),
                "ones": ones,
            }
        )
    return in_maps


def finish(res):
    """Host finish in f64: d1 soft-min per v from ssum, d2 exact per l.

    ssum rows r = 2*cm + h hold sum over c in {cm, cm+2} of the l-chunk
    partition sums for v-half h; total per v needs rows {h, 2+h}.
    """
    outs = []
    for cidx in range(N_CORES):
        ss = res.results[cidx]["ssum"].astype(np.float64)  # [4, B, 512]
        d2 = res.results[cidx]["d2c"].astype(np.float64)   # [P, B, CL]
        S = np.stack([ss[0] + ss[2], ss[1] + ss[3]], axis=1)  # [B, H, 512]
        d1 = SHIFT - np.log(S) / BETA                      # [B, H, 512]
        d2l = SHIFT - np.log(d2) / BETA                    # [P, B, CL]
        out = d1.mean(axis=(1, 2)) + d2l.mean(axis=(0, 2))
        outs.append(out.astype(np.float32))
    return np.concatenate(outs)


def kernel(video_feat, lang_feat):
    in_maps = make_in_maps(video_feat, lang_feat)
    res = _run(in_maps, trace=False)
    return finish(res).astype(np.float32)
